# revision 27
# baseline (speedup 1.0000x reference)
"""Trainium2 Bass kernel for nn_Enhanced_GAT_HSPA_76879914598912 (sparse_attention).

Mathematical collapse
---------------------
The reference's sparsemax-style ``soft_threshold`` subtracts the row max first,
so the sorted top values satisfy z_1 = 0 >= z_2 >= ... >= z_64 (all <= 0).
The support condition is  mask_k = [k <= csum_k / (z_k + 1e-8)]:

  * k = 1: ratio = 0/(0+eps) = 0 < 1  -> false.
  * k >= 2, z_k + eps > 0: ratio = csum_k/(+) <= 0 < k -> false.
  * k >= 2, z_k + eps < 0: ratio = |csum_k|/|z_k+eps|; since every |z_i| <= |z_k|,
    ratio <= k * |z_k|/(|z_k|-eps) which is < k unless ALL top-k values sit within
    k*1e-8 (= at most 6.4e-7) of the row max.  fp32 ties at that scale cannot occur
    for this data (row gaps are ~1e-1; verified min margin of (k - ratio) over all
    256*169 rows and all k is 0.9996 on the exact setup_inputs() data).

Hence supp == 1, tau == csum[0]/(1+eps) == 0, attn == relu(s - 0) == 0 identically
(s <= 0 everywhere).  Then agg = zf @ attn == 0 exactly, and the network reduces to

    out = res_scale * PReLU(BN(fu_w[:, 256:512] @ xf_flat + fu_b)) + xf_flat

i.e. a 1x1 conv (right half of fu_w) + inference-BN + PReLU + residual on xf only.
BN is folded into the conv weights/bias on the host; for res_scale >= 0 the outer
residual scale is folded in as well (PReLU is positively homogeneous).

Kernel strategy (pure data parallel, per the sharding hint)
-----------------------------------------------------------
xf [256, 256, 625] is sharded along batch across 8 cores (32 items each); the
small folded weight [256,256] and bias are replicated.  Per item and output
channel block m (2 blocks of 128):

    psum[m]  = conv terms accumulated on the TensorE (see variants below)
    fz       = Prelu(psum + bias_m, alpha)                     (ScalarE, from PSUM)
    out[m]   = fz + xf[m-block]                                (VectorE)

Items are processed in groups of 4 so each DMA moves >= 0.6 MB for good HBM
efficiency; input loads go through the SP HWDGE FIFO and output stores through
the ACT FIFO so stores never head-of-line-block loads.

Two program variants:
  * "bf16pair" (default): xf and the weight are shipped as (hi, lo) bf16 pairs
    (same bytes as fp32).  conv = wh@xh + wh@xl + wl@xh — 3 bf16 matmuls at
    1 cycle/row instead of fp32's 4 cycles/row, which moves the TensorE
    (~100us) under the DMA roofline (~115us/core).  The dropped wl@xl term and
    the residual reconstruction (fz + xh + xl) are both ~2^-18 relative.
    End-to-end: 5.1e-6 rel err, ~123us/core modeled.
  * "fp32": exact fp32 conv; TensorE-bound at ~140us busy (~158us total).
"""

import numpy as np

import concourse.bacc as bacc
import concourse.bass as bass
import concourse.mybir as mybir
import concourse.tile as tile
from concourse.bass_utils import run_bass_kernel_spmd

F32 = mybir.dt.float32

N_CORES = 8
B, C, H, W = 256, 256, 25, 25
NSP = H * W                  # 625
BPC = B // N_CORES           # 32 items per core
GROUP = 4                    # items per DMA group
N0 = 512                     # first matmul N-tile (PSUM bank limit for fp32)


def _build_program(
    n_items: int,
    alpha: float,
    res_mul: float | None,
    *,
    group: int = GROUP,
    xk_bufs: int = 3,
    ob_bufs: int = 4,
    fz_bufs: int = 4,
    ps_bufs: int = 4,
    out_engine: str = "sync",
) -> bass.Bass:
    """Emit the per-core Tile program.

    res_mul: None -> residual scale already folded into weights (res >= 0 path);
             float -> multiply the conv output by res_mul explicitly (res < 0).
    out_engine: which engine issues output DMAs ("sync" shares the SP HWDGE
                FIFO with input loads; "scalar" uses the ACT FIFO instead).
    """
    # Bacc (not raw Bass): its compile() runs move_matmul_waits_to_ldweights /
    # generate_event_semaphores, which TRN2 codegen needs (1 wait per inst).
    nc = bacc.Bacc("TRN2")
    xf_d = nc.dram_tensor("xf", [n_items, C, NSP], F32, kind="ExternalInput")
    wT_d = nc.dram_tensor("wT", [C, C], F32, kind="ExternalInput")      # [cin, cout]
    b_d = nc.dram_tensor("bias", [128, 2], F32, kind="ExternalInput")   # [p, m_block]
    out_d = nc.dram_tensor("out", [n_items, C, NSP], F32, kind="ExternalOutput")

    n_groups = (n_items + group - 1) // group
    out_eng = {"sync": nc.sync, "scalar": nc.scalar, "gpsimd": nc.gpsimd}[out_engine]

    with tile.TileContext(nc) as tc:
        with (
            tc.tile_pool(name="consts", bufs=1) as consts,
            tc.tile_pool(name="xk", bufs=xk_bufs) as xkp,
            tc.tile_pool(name="ps", bufs=ps_bufs, space="PSUM") as psp,
            tc.tile_pool(name="fz", bufs=fz_bufs) as fzp,
            tc.tile_pool(name="ob", bufs=ob_bufs) as obp,
        ):
            # Replicated constants: weights as lhsT tiles [cin-part, cout], bias.
            wk0 = consts.tile([128, C], F32)
            wk1 = consts.tile([128, C], F32)
            nc.sync.dma_start(wk0[:], wT_d[0:128, :])
            nc.sync.dma_start(wk1[:], wT_d[128:256, :])
            wk = (wk0, wk1)
            bt = consts.tile([128, 2], F32)
            nc.sync.dma_start(bt[:], b_d[:])

            for g in range(n_groups):
                b0 = g * group
                gsz = min(group, n_items - b0)
                xk0 = xkp.tile([128, gsz, NSP], F32, tag="xk0")
                xk1 = xkp.tile([128, gsz, NSP], F32, tag="xk1")
                nc.sync.dma_start(
                    xk0[:], xf_d[b0 : b0 + gsz, 0:128, :].rearrange("a p c -> p a c")
                )
                nc.sync.dma_start(
                    xk1[:], xf_d[b0 : b0 + gsz, 128:256, :].rearrange("a p c -> p a c")
                )
                xk = (xk0, xk1)

                for m in range(2):
                    pss = [
                        psp.tile([128, NSP], F32, name="ps", tag="ps")
                        for _ in range(gsz)
                    ]
                    # k-outer ordering -> one weight load per (group, m, k).
                    for k in range(2):
                        lhsT = wk[k][:, m * 128 : (m + 1) * 128]
                        for i in range(gsz):
                            nc.tensor.matmul(
                                pss[i][:, 0:N0],
                                lhsT,
                                xk[k][:, i, 0:N0],
                                start=(k == 0),
                                stop=(k == 1),
                            )
                            nc.tensor.matmul(
                                pss[i][:, N0:NSP],
                                lhsT,
                                xk[k][:, i, N0:NSP],
                                start=(k == 0),
                                stop=(k == 1),
                            )
                    ob = obp.tile([128, gsz, NSP], F32, tag="ob")
                    for i in range(gsz):
                        fz = fzp.tile([128, NSP], F32, tag="fz")
                        # Prelu honors the alpha operand on TRN2 walrus;
                        # Lrelu ignores it (fixed 0.01 leak) — HW-probed.
                        nc.scalar.activation(
                            fz[:],
                            pss[i][:, 0:NSP],
                            mybir.ActivationFunctionType.Prelu,
                            bias=bt[:, m : m + 1],
                            scale=1.0,
                            alpha=alpha,
                        )
                        if res_mul is not None:
                            nc.vector.tensor_scalar_mul(fz[:], fz[:], res_mul)
                        nc.vector.tensor_add(ob[:, i, :], fz[:], xk[m][:, i, :])
                    out_eng.dma_start(
                        out_d[b0 : b0 + gsz, m * 128 : (m + 1) * 128, :].rearrange(
                            "a p c -> p a c"
                        ),
                        ob[:],
                    )
    nc.finalize()  # Bacc: runs compile() (wait-splitting, reg alloc) + freeze
    return nc


def _build_program_bf16pair(
    n_items: int,
    alpha: float,
    res_mul: float | None,
    *,
    group: int = GROUP,
    xk_bufs: int = 3,
    ob_bufs: int = 4,
    fz_bufs: int = 4,
    ps_bufs: int = 4,
    out_engine: str = "sync",
    out_split: int = 1,
    group_layout: list[int] | None = None,
) -> bass.Bass:
    """bf16 hi/lo-pair variant: xf and the folded weight are shipped as
    (hi, lo) bf16 pairs (same total bytes as fp32).  The conv is computed as
    wh@xh + wh@xl + wl@xh (the dropped wl@xl term is ~2^-18 relative), giving
    3 bf16 matmuls at 1 cycle/row instead of fp32's 4 cycles/row - PE drops
    below the DMA roofline.  The residual is reconstructed as fz + xh + xl
    (error <= 2^-18 |xf|).
    """
    BF16 = mybir.dt.bfloat16
    nc = bacc.Bacc("TRN2")
    xh_d = nc.dram_tensor("xh", [n_items, C, NSP], BF16, kind="ExternalInput")
    xl_d = nc.dram_tensor("xl", [n_items, C, NSP], BF16, kind="ExternalInput")
    # All four weight half-tiles packed into one DMA:
    # cols [0:256]=wh k0, [256:512]=wh k1, [512:768]=wl k0, [768:1024]=wl k1.
    wpk_d = nc.dram_tensor("wpk", [128, 4 * C], BF16, kind="ExternalInput")
    b_d = nc.dram_tensor("bias", [128, 2], F32, kind="ExternalInput")
    out_d = nc.dram_tensor("out", [n_items, C, NSP], F32, kind="ExternalOutput")

    if group_layout is None:
        group_layout = []
        rem = n_items
        while rem > 0:
            gsz = min(group, rem)
            group_layout.append(gsz)
            rem -= gsz
    assert sum(group_layout) == n_items
    n_groups = len(group_layout)
    out_eng = {"sync": nc.sync, "scalar": nc.scalar, "gpsimd": nc.gpsimd}[out_engine]

    with tile.TileContext(nc) as tc:
        with (
            tc.tile_pool(name="consts", bufs=1) as consts,
            tc.tile_pool(name="xk", bufs=xk_bufs) as xkp,
            tc.tile_pool(name="ps", bufs=ps_bufs, space="PSUM") as psp,
            tc.tile_pool(name="fz", bufs=fz_bufs) as fzp,
            tc.tile_pool(name="ob", bufs=ob_bufs) as obp,
        ):
            def load_group(b0, gsz):
                xh0 = xkp.tile([128, gsz, NSP], BF16, name="xh0", tag="xh0")
                xh1 = xkp.tile([128, gsz, NSP], BF16, name="xh1", tag="xh1")
                xl0 = xkp.tile([128, gsz, NSP], BF16, name="xl0", tag="xl0")
                xl1 = xkp.tile([128, gsz, NSP], BF16, name="xl1", tag="xl1")
                for t, d_, lo in ((xh0, xh_d, 0), (xh1, xh_d, 128),
                                  (xl0, xl_d, 0), (xl1, xl_d, 128)):
                    nc.sync.dma_start(
                        t[:],
                        d_[b0 : b0 + gsz, lo : lo + 128, :].rearrange("a p c -> p a c"),
                    )
                return (xh0, xh1), (xl0, xl1)

            wpk = consts.tile([128, 4 * C], BF16)
            nc.sync.dma_start(wpk[:], wpk_d[:])
            wh = (wpk[:, 0:C], wpk[:, C : 2 * C])
            wl = (wpk[:, 2 * C : 3 * C], wpk[:, 3 * C : 4 * C])
            bt = consts.tile([128, 2], F32)
            nc.sync.dma_start(bt[:], b_d[:])

            b0 = 0
            for g in range(n_groups):
                gsz = group_layout[g]
                xh_t, xl_t = load_group(b0, gsz)

                for m in range(2):
                    pss = [
                        psp.tile([128, NSP], F32, name="ps", tag="ps")
                        for _ in range(gsz)
                    ]
                    # 3 bf16 terms per k-tile; wh reused across 2 consecutive
                    # terms to minimize weight reloads.
                    terms = [(0, wh, xh_t), (0, wh, xl_t), (0, wl, xh_t),
                             (1, wh, xh_t), (1, wh, xl_t), (1, wl, xh_t)]
                    for ti, (k, w_t, x_t) in enumerate(terms):
                        lhsT = w_t[k][:, m * 128 : (m + 1) * 128]  # slice of wpk AP
                        first = ti == 0
                        last = ti == len(terms) - 1
                        for i in range(gsz):
                            nc.tensor.matmul(
                                pss[i][:, 0:N0], lhsT, x_t[k][:, i, 0:N0],
                                start=first, stop=last,
                            )
                            nc.tensor.matmul(
                                pss[i][:, N0:NSP], lhsT, x_t[k][:, i, N0:NSP],
                                start=first, stop=last,
                            )
                    ob = obp.tile([128, gsz, NSP], F32, tag="ob")
                    for i in range(gsz):
                        fz = fzp.tile([128, NSP], F32, tag="fz")
                        nc.scalar.activation(
                            fz[:],
                            pss[i][:, 0:NSP],
                            mybir.ActivationFunctionType.Prelu,
                            bias=bt[:, m : m + 1],
                            scale=1.0,
                            alpha=alpha,
                        )
                        if res_mul is not None:
                            nc.vector.tensor_scalar_mul(fz[:], fz[:], res_mul)
                        nc.vector.tensor_add(fz[:], fz[:], xh_t[m][:, i, :])
                        nc.vector.tensor_add(ob[:, i, :], fz[:], xl_t[m][:, i, :])
                        # Last group only: drain finished items immediately so
                        # the end-of-kernel bubble is one item, not the group.
                        split = out_split if g == n_groups - 1 else 1
                        chunk = max(1, gsz // split) if split > 1 else gsz
                        if (i + 1) % chunk == 0:
                            c0 = i + 1 - chunk
                            out_eng.dma_start(
                                out_d[
                                    b0 + c0 : b0 + i + 1, m * 128 : (m + 1) * 128, :
                                ].rearrange("a p c -> p a c"),
                                ob[:, c0 : i + 1, :],
                            )
                b0 += gsz
    nc.finalize()
    return nc


def _split_bf16_pair(a: np.ndarray):
    """a (fp32) -> (hi, lo) bf16 with hi + lo ~= a  (error <= 2^-18 |a|)."""
    import ml_dtypes

    hi = a.astype(ml_dtypes.bfloat16)
    lo = (a - hi.astype(np.float32)).astype(ml_dtypes.bfloat16)
    return hi, lo


def _fold_params(inputs: dict[str, np.ndarray]):
    """Fold BN (+ residual scale when >= 0) into the right half of fu_w."""
    fu_w = np.asarray(inputs["fu_w"], np.float32)
    fu_b = np.asarray(inputs["fu_b"], np.float32)
    fu_g = np.asarray(inputs["fu_g"], np.float32)
    fu_be = np.asarray(inputs["fu_be"], np.float32)
    fu_m = np.asarray(inputs["fu_m"], np.float32)
    fu_v = np.asarray(inputs["fu_v"], np.float32)
    alpha = float(np.asarray(inputs["fu_a"]).reshape(-1)[0])
    res = float(np.asarray(inputs["res_scale"]).reshape(-1)[0])

    scale = (fu_g / np.sqrt(fu_v + np.float32(1e-5))).astype(np.float32)
    W2 = (fu_w[:, C:] * scale[:, None]).astype(np.float32)          # [cout, cin]
    bfold = ((fu_b - fu_m) * scale + fu_be).astype(np.float32)      # [cout]
    if res >= 0.0:
        W2 = (W2 * np.float32(res)).astype(np.float32)
        bfold = (bfold * np.float32(res)).astype(np.float32)
        res_mul = None
    else:
        res_mul = res
    wT = np.ascontiguousarray(W2.T)                                 # [cin, cout]
    bias2 = np.ascontiguousarray(bfold.reshape(2, 128).T)           # [128, 2]
    return wT, bias2, alpha, res_mul


# Last BassKernelResults (exec_time_ns etc.) for harness inspection.
last_results = None

# "bf16pair" (default): 3x bf16 matmuls on hi/lo split operands — PE under the
# DMA roofline, ~4e-6 extra error.  "fp32": exact-fp32 conv (PE-bound, ~1.4x
# slower) kept as fallback.
VARIANT = "bf16pair"

# Tuned via TimelineSim sweep; see test.py for the HW-time estimate.
# Small first/last groups ramp the DMA pipeline faster and drain the tail
# sooner; the middle runs at 1.25MB-per-DMA steady state.
BUILD_KWARGS = dict(
    xk_bufs=3, ob_bufs=6, fz_bufs=6, out_engine="scalar", out_split=2,
    group_layout=[2, 2] + [4] * 6 + [2, 2],
)


def kernel(**inputs: np.ndarray) -> np.ndarray:
    global last_results
    xf = np.ascontiguousarray(np.asarray(inputs["xf"], np.float32)).reshape(B, C, NSP)
    wT, bias2, alpha, res_mul = _fold_params(inputs)

    if VARIANT == "bf16pair":
        nc = _build_program_bf16pair(BPC, alpha, res_mul, **BUILD_KWARGS)
        xh, xl = _split_bf16_pair(xf)
        wh, wl = _split_bf16_pair(wT)
        # Pack the 4 weight half-tiles [cin-part, cout] into one [128, 1024].
        wpk = np.concatenate(
            [wh[0:128], wh[128:256], wl[0:128], wl[128:256]], axis=1
        )
        wpk = np.ascontiguousarray(wpk)
        in_maps = [
            {
                "xh": np.ascontiguousarray(xh[i * BPC : (i + 1) * BPC]),
                "xl": np.ascontiguousarray(xl[i * BPC : (i + 1) * BPC]),
                "wpk": wpk,
                "bias": bias2,
            }
            for i in range(N_CORES)
        ]
    else:
        nc = _build_program(BPC, alpha, res_mul)
        in_maps = [
            {
                "xf": np.ascontiguousarray(xf[i * BPC : (i + 1) * BPC]),
                "wT": wT,
                "bias": bias2,
            }
            for i in range(N_CORES)
        ]
    kres = run_bass_kernel_spmd(nc, in_maps, core_ids=list(range(N_CORES)))
    last_results = kres
    out = np.concatenate([np.asarray(r["out"]) for r in kres.results], axis=0)
    return out.reshape(B, C, H, W).astype(np.float32)
